# revision 64
# baseline (speedup 1.0000x reference)
"""Trainium2 Bass kernel for nn_BHSDuelingDQN (gnn_message_passing).

Math notes (validated vs reference to fp32 precision):
  - The edge MLP input is ones(E,1), so every edge shares one theta [F,OUT]:
        theta = (relu(w1[0]+b1) @ w2 + b2).reshape(F, OUT)
  - edge_index values live in [0, N), so the gather/scatter-add only touches
    batch 0 of flat=[B*N,F].  With C[s,t] = #edges(src=s, tgt=t):
        agg(batch0) = C^T @ (x[0] @ theta)
    which turns the whole message passing into dense matmuls.  C is built on
    the host from edge_index (pure index bookkeeping; all FLOPs with x /
    theta / weights stay on device).

Sharding: phase 1 is node-sharded (each of 8 cores owns 32 of 256 nodes and
computes partial pre-activations of adv/v1 for all 1024 batches over its
4096 feature rows).  Phase 2 is batch-sharded (each core sums the 8 partials
for its 128 batches and runs the small value-head + dueling combine).  The
host only slices / concatenates / transposes arrays between phases.

Phase-1 schedule: the PE is the bottleneck (conv 32768 + accumulate 65536
cycles per core at full f32r rate).  Everything else is built around keeping
the PE gapless and fully ramped:
  - warm-up matmuls on memset/identity data bridge the initial DMA latency
    AND the p-state ramp (PE runs 2x slower for its first ~3us of busy time).
  - one flat conv/acc pipeline across three batch sweeps (512/256/256, the
    last narrow so the final drain+DMA tail is short); convs run 3 nodes
    ahead of the accumulates so the relu (alternating Act/DVE; GPSIMD cannot
    read PSUM) is never on the PE critical path.
  - head biases are folded into the drains; the batch-0 column (the only one
    with a scatter-add contribution) is patched into each feat tile from a
    precomputed feat0 column.  Phase-1 partials go to HBM as fp16.
Phase 2 folds the dueling mean/expand structure into the weights on the host
(pure index structure), so each 64-batch chunk is 3 fp16 tree-adds + relu +
10 small matmuls; the 8-partial reduction and all model FLOPs stay on device.

HW-ISA notes learned the hard way: GPSIMD cannot access PSUM; fp32r matmuls
need moving-dim >= 256 for full rate and reject 1-column outputs; matmul
fmap/weight must share their SBUF start partition, which must equal the PE
row tile position; f32->f32r bitcasts of engine outputs are rejected (only
engines and DMA may produce f32r); f32r/f16 memsets are invalid ISA.
"""

import os
from contextlib import ExitStack

import numpy as np

import concourse.bacc as bacc
import concourse.bass as bass
import concourse.mybir as mybir
import concourse.tile as tile
from concourse import masks
from concourse.bass_utils import run_bass_kernel_spmd  # noqa: F401  (contract)

F32 = mybir.dt.float32
F32R = mybir.dt.float32r
F16 = mybir.dt.float16

B, N, F, E, OUT, NDIV, PER = 1024, 256, 8, 1024, 128, 64, 3
NADV = NDIV * PER            # 192
AV = NADV + 64               # 256 fused output cols of phase 1 (adv | v1)
M = 8                        # cores
NPC = N // M                 # 32 nodes per core
MODE = os.environ.get("BASS_KERNEL_MODE", "f32r")  # kept for test.py compat

WARM_N = int(os.environ.get("BASS_WARM_N", "8"))

# rwx column map (f32r, first DMA on the sync queue: everything the sweep's
# start needs — root weight replicas, conv bias, head-bias columns, edge MLP)
RX_RW = 0                     # 128 cols: root_w at partitions 32j..32j+8
RX_CB, RX_BR0, RX_BR1, RX_W1T, RX_B1T = 128, 129, 130, 131, 132
RX_X0TL = 133                 # 32 cols [8, 32]: x[0] local nodes, transposed
RX_COLS = 165

# pa0 column map (f32, scalar queue, needed only by the phase-0 chain)
PA_B2T = 0                    # 8 cols  [128, 8] = b2.reshape(F, OUT).T
PA_X0T = 8                    # 256 cols [8, 256] = x[0].T
PA_C = 264                    # 64 cols  [128, 2, 32] edge-count matrix
PA0_COLS = PA_C + 2 * NPC     # 328

# batch sweeps: (xw column base, width, output batch base); the last one is
# narrow so the final drain + output DMA tail is short.  Batch 0 (the only
# one with a scatter-add contribution) sits at sweep 1's first column.
SWEEPS = ((0, 512, 512), (1536, 256, 0), (1792, 256, 256))

_build_cache = {}


def _build_phase1(repeat=1):
    nc = bacc.Bacc("TRN2")

    rwx_d = nc.dram_tensor("rwx", [128, RX_COLS], F32R, kind="ExternalInput")
    pa0_d = nc.dram_tensor("pa0", [128, PA0_COLS], F32, kind="ExternalInput")
    pw2_d = nc.dram_tensor("pw2", [64, F * OUT], F32, kind="ExternalInput")
    xw_d = nc.dram_tensor("xw", [128, 8, 2048], F32R, kind="ExternalInput")
    pt_d = nc.dram_tensor("pt", [128, 2, B], F16, kind="ExternalOutput")

    with tile.TileContext(nc) as tc:
      for rep in range(repeat):
        with ExitStack() as ctx:
            const = ctx.enter_context(tc.tile_pool(name=f"const{rep}", bufs=1))

            # tiny device-generated tensors first: no DMA deps
            wsmall_sb = const.tile([128, 16], F32, name="wsmall_sb")
            nc.gpsimd.memset(wsmall_sb, 0.0)
            ident_sb = const.tile([128, 128], F32, name="ident_sb")
            masks.make_identity(nc, ident_sb)

            # DMAs in consumption order
            rwx_sb = const.tile([128, RX_COLS], F32R, name="rwx_sb")
            nc.sync.dma_start(out=rwx_sb, in_=rwx_d[:])
            xw_sb = const.tile([128, 8, 2048], F32R, name="xw_sb")
            nc.sync.dma_start(out=xw_sb[:, 0, 0:512], in_=xw_d[:, 0, 0:512])
            pa0_sb = const.tile([128, PA0_COLS], F32, name="pa0_sb")
            nc.scalar.dma_start(out=pa0_sb, in_=pa0_d[:])
            nc.sync.dma_start(out=xw_sb[:, 0, 512:1024], in_=xw_d[:, 0, 512:1024])
            pw2_sb = const.tile([64, F * OUT], F32, name="pw2_sb")
            nc.scalar.dma_start(out=pw2_sb, in_=pw2_d[:])
            nc.sync.dma_start(out=xw_sb[:, 0, 1024:1536], in_=xw_d[:, 0, 1024:1536])
            nc.sync.dma_start(out=xw_sb[:, 1, 0:512], in_=xw_d[:, 1, 0:512])
            nc.sync.dma_start(out=xw_sb[:, 1, 512:1024], in_=xw_d[:, 1, 512:1024])
            nc.sync.dma_start(out=xw_sb[:, 1, 1024:1536], in_=xw_d[:, 1, 1024:1536])
            for g in range(2, 8):
                nc.sync.dma_start(out=xw_sb[:, g, 0:512], in_=xw_d[:, g, 0:512])
                nc.sync.dma_start(
                    out=xw_sb[:, g, 512:1536], in_=xw_d[:, g, 512:1536]
                )
            for g in range(8):
                nc.sync.dma_start(
                    out=xw_sb[:, g, 1536:2048], in_=xw_d[:, g, 1536:2048]
                )

            def pa(col, ncols, nrows=128):
                return pa0_sb[0:nrows, col : col + ncols]

            def rx(col, ncols, nrows=128):
                # f32 bitcast view (f32r is only needed for matmul operands)
                return rwx_sb[0:nrows, col : col + ncols].bitcast(F32)

            cb_ap = rx(RX_CB, 1)

            # small SBUF intermediates
            h_sb = const.tile([64, 1], F32, name="h_sb")
            thT_sb = const.tile([128, F], F32, name="thT_sb")
            th_sb = const.tile([F, OUT], F32, name="th_sb")
            x0th_sb = const.tile([128, 2, OUT], F32, name="x0th_sb")
            feat0_sb = const.tile([128, NPC], F32R, name="feat0_sb")

            acc_pool = ctx.enter_context(
                tc.tile_pool(name=f"accp{rep}", bufs=2, space="PSUM")
            )
            conv_pool = ctx.enter_context(
                tc.tile_pool(name=f"convp{rep}", bufs=3, space="PSUM")
            )
            p0_pool = ctx.enter_context(
                tc.tile_pool(name=f"p0p{rep}", bufs=1, space="PSUM")
            )
            feat_pool = ctx.enter_context(tc.tile_pool(name=f"featp{rep}", bufs=6))
            out_pool = ctx.enter_context(tc.tile_pool(name=f"outp{rep}", bufs=1))
            out_sb = out_pool.tile([128, 2, B], F16, name="out_sb")

            # ---- PE warm-up: no-dep fp32 matmuls (4 cycles/row) bridge the
            # initial DMA latency and the p-state ramp; the first few are
            # narrow so the PE starts the moment the 16-col memset lands
            for k in range(5):
                warm_ps = conv_pool.tile(
                    [16, 16], F32, name="warm_ps", tag="conv_ps"
                )
                nc.tensor.matmul(warm_ps, wsmall_sb[:, 0:16], wsmall_sb)
            for k in range(WARM_N):
                warm_ps = conv_pool.tile(
                    [128, 128], F32, name="warm_ps", tag="conv_ps"
                )
                nc.tensor.matmul(warm_ps, ident_sb, ident_sb)

            def wc_ap(n, m):
                base = 512 + (n % 4) * 256 + m * 128
                return xw_sb[:, n // 4, base : base + 128]

            RELU_ENG = ("act", "dve")

            def emit_conv(n, xbase, width, agg):
                j = n % 4
                conv_ps = conv_pool.tile(
                    [128, width], F32, name="conv_ps", tag="conv_ps"
                )
                nc.tensor.matmul(
                    conv_ps,
                    rwx_sb[32 * j : 32 * j + F, RX_RW : RX_RW + OUT],
                    xw_sb[32 * j : 32 * j + F, n // 4, xbase : xbase + width],
                    tile_position=(32 * j, 0),
                )
                feat_sb = feat_pool.tile([128, width], F32R, name="feat_sb")
                eng = RELU_ENG[n % 2]
                if eng == "act":
                    nc.scalar.activation(
                        feat_sb,
                        conv_ps,
                        mybir.ActivationFunctionType.Relu,
                        bias=cb_ap,
                    )
                elif eng == "dve":
                    nc.vector.tensor_scalar(
                        feat_sb, conv_ps, cb_ap, 0.0,
                        mybir.AluOpType.add, mybir.AluOpType.max,
                    )
                if agg:
                    # batch 0 (column 0) gets the precomputed scatter-add
                    # corrected feature column for this node
                    if n % 2 == 0:
                        nc.vector.tensor_copy(
                            feat_sb[:, 0:1], feat0_sb[:, n : n + 1]
                        )
                    else:
                        nc.scalar.activation(
                            feat_sb[:, 0:1],
                            feat0_sb[:, n : n + 1],
                            mybir.ActivationFunctionType.Copy,
                        )
                return feat_sb

            acc_tiles = {}

            def drain(si):
                xbase, width, bbase = SWEEPS[si]
                sl = slice(bbase, bbase + width)
                acc_ps = acc_tiles[si]
                nc.vector.tensor_scalar_add(
                    out_sb[:, 0, sl], acc_ps[0], rx(RX_BR0, 1)
                )
                nc.scalar.activation(
                    out_sb[:, 1, sl],
                    acc_ps[1],
                    mybir.ActivationFunctionType.Identity,
                    bias=rx(RX_BR1, 1),
                )
                nc.sync.dma_start(out=pt_d[:, :, sl], in_=out_sb[:, :, sl])

            def run_sweeps(inject=()):
                # one flat conv/acc pipeline across all sweeps: convs run a
                # few nodes ahead so sweep boundaries leave no PE gap
                inject = dict(inject)
                pend = []

                def emit_acc(si, n, feat_sb):
                    for m in range(2):
                        nc.tensor.matmul(
                            acc_tiles[si][m],
                            wc_ap(n, m),
                            feat_sb,
                            start=(n == 0),
                            stop=(n == NPC - 1),
                        )
                    if n == NPC - 1:
                        drain(si)

                events = [(si, n) for si in range(len(SWEEPS))
                          for n in range(NPC)]
                for gi, (si, n) in enumerate(events):
                    if n == 0:
                        acc_tiles[si] = [
                            acc_pool.tile(
                                [128, SWEEPS[si][1]], F32,
                                name=f"acc{m}_{si}", tag=f"acc{m}",
                            )
                            for m in range(2)
                        ]
                    pend.append(
                        (si, n, emit_conv(n, SWEEPS[si][0], SWEEPS[si][1],
                                          si == 1))
                    )
                    if gi in inject:
                        inject[gi]()
                    if len(pend) > 3:
                        psi, pn, pf = pend.pop(0)
                        emit_acc(psi, pn, pf)
                for psi, pn, pf in pend:
                    emit_acc(psi, pn, pf)

            # ---- phase 0 steps: theta, x0@theta, aggT -------------------
            # interleaved into the h=1 sweep so the serial chain (with its
            # cross-engine semaphore latencies) hides behind conv/acc work
            def p0_theta():
                nc.vector.tensor_scalar(
                    h_sb, rx(RX_W1T, 1, 64), rx(RX_B1T, 1, 64), 0.0,
                    mybir.AluOpType.add, mybir.AluOpType.max,
                )
                thT_ps = p0_pool.tile([128, F], F32, name="thT_ps", tag="p0")
                for f in range(F):
                    nc.tensor.matmul(
                        thT_ps[:, f : f + 1],
                        pw2_sb[:, f * OUT : (f + 1) * OUT],
                        h_sb,
                    )
                nc.vector.tensor_add(thT_sb, thT_ps, pa(PA_B2T, F))

            def p0_th():
                th_ps = p0_pool.tile([F, OUT], F32, name="th_ps", tag="p0")
                nc.tensor.transpose(th_ps, thT_sb[:, 0:F], ident_sb)
                nc.vector.tensor_copy(th_sb, th_ps)

            def p0_x0th(s):
                x0th_ps = p0_pool.tile(
                    [128, OUT], F32, name=f"x0th_ps{s}", tag="p0"
                )
                nc.tensor.matmul(
                    x0th_ps, pa(PA_X0T + s * 128, 128, F), th_sb
                )
                nc.vector.tensor_copy(x0th_sb[:, s, :], x0th_ps)

            def p0_feat0():
                agg_ps = p0_pool.tile([128, NPC], F32, name="agg_ps", tag="p0")
                for s in range(2):
                    nc.tensor.matmul(
                        agg_ps,
                        x0th_sb[:, s, :],
                        pa(PA_C + s * NPC, NPC),
                        start=(s == 0),
                        stop=False,
                    )
                nc.tensor.matmul(
                    agg_ps,
                    rwx_sb[0:F, RX_RW : RX_RW + OUT],
                    rwx_sb[0:F, RX_X0TL : RX_X0TL + NPC],
                    start=False,
                    stop=True,
                )
                nc.scalar.activation(
                    feat0_sb,
                    agg_ps,
                    mybir.ActivationFunctionType.Relu,
                    bias=cb_ap,
                )

            # sweep 0 carries the phase-0 chain; sweep 1 needs aggT ready
            run_sweeps(inject={
                8: p0_theta,
                11: p0_th,
                14: lambda: p0_x0th(0),
                16: lambda: p0_x0th(1),
                18: p0_feat0,
            })

    nc.finalize()
    return nc


# phase-2 folded-weight layout (all structural matrices pre-multiplied into
# the weights on the host; the dueling mean/expand matrices are pure index
# structure so this is weight re-indexing, not model compute)
PF_V2W = 0      # [65, 64]  v2w with v2b as K row 64
PF_A0 = 64      # [128, 128] I + MGA@EM0
PF_B0 = 192     # [64, 128]  MGB@EM0
PF_C0 = 320     # [64, 128]  V3@EM0
PF_CB0 = 448    # [1, 128]   v3b@EM0
PF_A1 = 576     # [128, 64]  MGA@EM1
PF_B1 = 640     # [64, 64]   I + MGB@EM1
PF_C1 = 704     # [64, 64]   V3@EM1
PF_CB1 = 768    # [1, 64]    v3b@EM1
PF_COLS = 832
BT = B // M      # 128 batches per core
# even batch chunks, each a contiguous dram tensor (strided chunk slices of
# one flat tensor fall under the 512B inner-run threshold and pay a 2x DMA
# latency multiplier)
P2_CHUNKS = ((0, 64), (64, 64))


def _build_phase2(repeat=1):
    nc = bacc.Bacc("TRN2")

    parts0_d = nc.dram_tensor("parts0", [128, 3, M, 64], F16, kind="ExternalInput")
    parts1_d = nc.dram_tensor("parts1", [128, 3, M, 64], F16, kind="ExternalInput")
    pf_d = nc.dram_tensor("pf", [128, PF_COLS], F16, kind="ExternalInput")
    ot_d = nc.dram_tensor("ot", [128, 2, BT], F32, kind="ExternalOutput")

    with tile.TileContext(nc) as tc:
      for rep in range(repeat):
        with ExitStack() as ctx:
            const = ctx.enter_context(tc.tile_pool(name=f"c2_{rep}", bufs=1))

            ones_sb = const.tile([128, BT], F16, name="ones_sb")
            nc.gpsimd.memset(ones_sb, 1.0)

            pp0_sb = const.tile([128, 3, M, 64], F16, name="pp0_sb")
            pp1_sb = const.tile([128, 3, M, 64], F16, name="pp1_sb")
            pf_sb = const.tile([128, PF_COLS], F16, name="pf_sb")
            nc.sync.dma_start(out=pp0_sb, in_=parts0_d[:])
            nc.scalar.dma_start(out=pf_sb, in_=pf_d[:])
            nc.sync.dma_start(out=pp1_sb, in_=parts1_d[:])

            def pf(col, ncols, nrows=128):
                return pf_sb[0:nrows, col : col + ncols]

            work = ctx.enter_context(tc.tile_pool(name=f"work{rep}", bufs=2))
            psum = ctx.enter_context(
                tc.tile_pool(name=f"psum{rep}", bufs=2, space="PSUM")
            )
            ot_sb = const.tile([128, 2, BT], F32, name="ot_sb")
            nc.gpsimd.memset(ot_sb[64:128, 1, :], 0.0)

            for (b0, bw), pp_t in zip(P2_CHUNKS, (pp0_sb, pp1_sb)):
                pp = pp_t[:]
                bsl = slice(b0, b0 + bw)
                eng = nc.vector
                # sum the 8 partials (tree on one engine: no sem hops;
                # biases already folded in by phase 1), then one relu
                s1 = work.tile([128, 3, 4, bw], F16, name="s1")
                eng.tensor_add(s1, pp[:, :, 0:4, :], pp[:, :, 4:8, :])
                s2 = work.tile([128, 3, 2, bw], F16, name="s2")
                eng.tensor_add(s2, s1[:, :, 0:2, :], s1[:, :, 2:4, :])
                r_all = work.tile([128, 3, bw], F16, name="r_all")
                s3 = work.tile([128, 3, bw], F16, name="s3")
                eng.tensor_add(s3, s2[:, :, 0, :], s2[:, :, 1, :])
                eng.tensor_scalar_max(r_all, s3, 0.0)
                ar0 = r_all[:, 0, :]
                ar1 = r_all[0:64, 1, :]
                v1r = r_all[0:64, 2, :]

                # v2 = relu(v1 @ v2w + v2b)
                v2_ps = psum.tile([64, bw], F32, name="v2_ps")
                nc.tensor.matmul(
                    v2_ps, pf(PF_V2W, 64, 64), v1r, start=True, stop=False
                )
                nc.tensor.matmul(
                    v2_ps, pf_sb[64:65, PF_V2W : PF_V2W + 64], ones_sb[64:65, 0:bw],
                    start=False, stop=True, tile_position=(64, 0),
                )
                v2r = work.tile([64, bw], F16, name="v2r")
                eng.tensor_scalar_max(v2r, v2_ps, 0.0)

                # folded dueling combine: out0 = A0^T ar0 + B0^T ar1
                #                              + C0^T v2r + cb0^T ones
                o0_ps = psum.tile([128, bw], F32, name="o0_ps")
                nc.tensor.matmul(o0_ps, pf(PF_A0, 128), ar0, start=True, stop=False)
                nc.tensor.matmul(o0_ps, pf(PF_B0, 128, 64), ar1, start=False, stop=False)
                nc.tensor.matmul(o0_ps, pf(PF_C0, 128, 64), v2r, start=False, stop=False)
                nc.tensor.matmul(
                    o0_ps, pf(PF_CB0, 128, 1), ones_sb[0:1, 0:bw],
                    start=False, stop=True, tile_position=(0, 0),
                )
                o1_ps = psum.tile([64, bw], F32, name="o1_ps")
                nc.tensor.matmul(o1_ps, pf(PF_A1, 64), ar0, start=True, stop=False)
                nc.tensor.matmul(o1_ps, pf(PF_B1, 64, 64), ar1, start=False, stop=False)
                nc.tensor.matmul(o1_ps, pf(PF_C1, 64, 64), v2r, start=False, stop=False)
                nc.tensor.matmul(
                    o1_ps, pf(PF_CB1, 64, 1), ones_sb[0:1, 0:bw],
                    start=False, stop=True, tile_position=(0, 0),
                )
                nc.scalar.activation(
                    ot_sb[:, 0, bsl], o0_ps, mybir.ActivationFunctionType.Copy
                )
                eng.tensor_copy(ot_sb[0:64, 1, bsl], o1_ps)
                nc.sync.dma_start(out=ot_d[:, :, bsl], in_=ot_sb[:, :, bsl])

    nc.finalize()
    return nc


def _get_programs(mode=None, repeat=1):
    key = repeat
    if key not in _build_cache:
        _build_cache[key] = (_build_phase1(repeat), _build_phase2(repeat))
    return _build_cache[key]


def _prep_phase1_inputs(inputs, mode=None):
    x = np.ascontiguousarray(np.asarray(inputs["x"], np.float32))
    ei = np.asarray(inputs["edge_index"]).astype(np.int64)
    w1 = np.asarray(inputs["w1"], np.float32)
    b1 = np.asarray(inputs["b1"], np.float32)
    w2 = np.asarray(inputs["w2"], np.float32)
    b2 = np.asarray(inputs["b2"], np.float32)
    root_w = np.asarray(inputs["root_w"], np.float32)
    conv_b = np.asarray(inputs["conv_b"], np.float32)
    adv_w = np.asarray(inputs["adv_w"], np.float32)
    v1w = np.asarray(inputs["v1w"], np.float32)
    adv_b = np.asarray(inputs["adv_b"], np.float32)
    v1b = np.asarray(inputs["v1b"], np.float32)

    src_i, tgt_i = ei[0], ei[1]
    wfull = np.concatenate([adv_w, v1w], axis=1)  # [32768, 256]

    rwx = np.zeros((128, RX_COLS), np.float32)
    for j in range(4):
        rwx[32 * j : 32 * j + F, RX_RW : RX_RW + OUT] = root_w
    rwx[:, RX_CB] = conv_b
    rwx[0:64, RX_W1T] = w1.reshape(64)
    rwx[0:64, RX_B1T] = b1

    pa0 = np.zeros((128, PA0_COLS), np.float32)
    pa0[:, PA_B2T : PA_B2T + F] = b2.reshape(F, OUT).T
    pa0[0:F, PA_X0T : PA_X0T + N] = x[0].T

    brow = np.concatenate([adv_b, v1b])          # [256]

    in_maps = []
    for c in range(M):
        rwc = rwx.copy()
        if c == 0:
            rwc[:, RX_BR0] = brow[0:128]
            rwc[:, RX_BR1] = brow[128:256]
        rwc[0:F, RX_X0TL : RX_X0TL + NPC] = x[0, NPC * c : NPC * (c + 1), :].T
        pac = pa0.copy()
        # edge-count matrix for this core's 32 target nodes
        cmat = np.zeros((N, NPC), np.float32)
        sel = (tgt_i >= NPC * c) & (tgt_i < NPC * (c + 1))
        np.add.at(cmat, (src_i[sel], tgt_i[sel] - NPC * c), 1.0)
        pac[:, PA_C : PA_C + NPC] = cmat[0:128]
        pac[:, PA_C + NPC : PA_C + 2 * NPC] = cmat[128:256]

        xw = np.zeros((128, 8, 2048), np.float32)
        xc = x[:, NPC * c : NPC * (c + 1), :]           # [B, 32, 8]
        xr = xc.transpose(1, 2, 0)                      # [32, 8, B]
        for j in range(4):
            # node 4g+j sits at partitions 32j..32j+F of group g
            xw[32 * j : 32 * j + F, :, 0:512] = xr[j::4, :, 512:1024].transpose(
                1, 0, 2
            )
            xw[32 * j : 32 * j + F, :, 1536:2048] = xr[j::4, :, 0:512].transpose(
                1, 0, 2
            )
        rows = wfull[4096 * c : 4096 * (c + 1)]         # [4096, 256]
        xw[:, :, 512:1536] = (
            rows.reshape(8, 4, 128, AV)
            .transpose(2, 0, 1, 3)
            .reshape(128, 8, 1024)
        )
        in_maps.append({"rwx": rwc, "pa0": pac, "pw2": w2, "xw": xw})
    return in_maps


def _prep_phase2_inputs(inputs, pts):
    v2w = np.asarray(inputs["v2w"], np.float32)
    v2b = np.asarray(inputs["v2b"], np.float32)
    v3w = np.asarray(inputs["v3w"], np.float32)
    v3b = np.asarray(inputs["v3b"], np.float32)

    # structural dueling matrices (index structure only)
    dp = np.arange(NADV)
    mg = np.zeros((NADV, NDIV), np.float32)
    mg[dp, dp // PER] = -1.0 / PER           # negated group-mean matrix
    em = np.zeros((NDIV, NADV), np.float32)  # expand d -> (d,p)
    em[dp // PER, dp] = 1.0
    em0, em1 = em[:, :128], em[:, 128:]
    mga, mgb = mg[:128], mg[128:]

    pfm = np.zeros((128, PF_COLS), np.float32)
    pfm[0:64, PF_V2W : PF_V2W + 64] = v2w
    pfm[64, PF_V2W : PF_V2W + 64] = v2b
    pfm[:, PF_A0 : PF_A0 + 128] = np.eye(128, dtype=np.float32) + mga @ em0
    pfm[0:64, PF_B0 : PF_B0 + 128] = mgb @ em0
    pfm[0:64, PF_C0 : PF_C0 + 128] = v3w @ em0
    pfm[0, PF_CB0 : PF_CB0 + 128] = v3b @ em0
    pfm[:, PF_A1 : PF_A1 + 64] = mga @ em1
    pfm[0:64, PF_B1 : PF_B1 + 64] = np.eye(64, dtype=np.float32) + mgb @ em1
    pfm[0:64, PF_C1 : PF_C1 + 64] = v3w @ em1
    pfm[0, PF_CB1 : PF_CB1 + 64] = v3b @ em1
    pfm = pfm.astype(np.float16)

    in_maps = []
    for c in range(M):
        bsl = slice(BT * c, BT * (c + 1))
        stk = np.stack([p[:, bsl] for p in pts])              # [i, 256, BT]
        parts = np.zeros((128, 3, M, BT), np.float16)
        parts[:, 0] = stk[:, 0:128, :].transpose(1, 0, 2)
        parts[0:64, 1] = stk[:, 128:NADV, :].transpose(1, 0, 2)
        parts[0:64, 2] = stk[:, NADV:AV, :].transpose(1, 0, 2)
        in_maps.append({
            "parts0": np.ascontiguousarray(parts[:, :, :, 0:64]),
            "parts1": np.ascontiguousarray(parts[:, :, :, 64:128]),
            "pf": pfm,
        })
    return in_maps


class _Runner:
    """Cached PJRT executor for one Bass program across the 8 cores.

    Mirrors bass2jax.run_bass_via_pjrt but keeps the jitted callable so
    repeat calls don't re-trace/re-lower, enabling benchmarking.
    """

    def __init__(self, nc):
        import jax
        from jax.sharding import Mesh, PartitionSpec, NamedSharding
        from jax.experimental.shard_map import shard_map
        from concourse import bass2jax

        bass2jax.install_neuronx_cc_hook()
        self.jax = jax
        self.nc = nc
        partition_name = (
            nc.partition_id_tensor.name if nc.partition_id_tensor else None
        )
        in_names, out_names, out_avals, zero_shapes = [], [], [], []
        for alloc in nc.m.functions[0].allocations:
            if not isinstance(alloc, mybir.MemoryLocationSet):
                continue
            name = alloc.memorylocations[0].name
            if alloc.kind == "ExternalInput":
                if name != partition_name:
                    in_names.append(name)
            elif alloc.kind == "ExternalOutput":
                shape = tuple(alloc.tensor_shape)
                dtype = mybir.dt.np(alloc.dtype)
                out_names.append(name)
                out_avals.append(jax.core.ShapedArray(shape, dtype))
                zero_shapes.append((shape, dtype))
        self.in_names, self.out_names = in_names, out_names
        self.out_avals, self.zero_shapes = out_avals, zero_shapes
        n_params, n_outs = len(in_names), len(out_names)
        self.n_params = n_params

        bind_names = in_names + out_names
        if partition_name is not None:
            bind_names = bind_names + [partition_name]

        def _body(*args):
            operands = list(args)
            if partition_name is not None:
                operands.append(bass2jax.partition_id_tensor())
            outs = bass2jax._bass_exec_p.bind(
                *operands,
                out_avals=tuple(out_avals),
                in_names=tuple(bind_names),
                out_names=tuple(out_names),
                lowering_input_output_aliases=(),
                sim_require_finite=True,
                sim_require_nnan=True,
                nc=nc,
            )
            return tuple(outs)

        devices = jax.devices()[:M]
        self.mesh = Mesh(np.asarray(devices), ("core",))
        spec = PartitionSpec("core")
        self.sharding = NamedSharding(self.mesh, spec)
        donate = tuple(range(n_params, n_params + n_outs))
        self.fn = jax.jit(
            shard_map(
                _body,
                mesh=self.mesh,
                in_specs=(spec,) * (n_params + n_outs),
                out_specs=(spec,) * n_outs,
                check_rep=False,
            ),
            donate_argnums=donate,
            keep_unused=True,
        )

    def _concat_inputs(self, in_maps):
        return [
            np.concatenate([np.asarray(m[name]) for m in in_maps], axis=0)
            for name in self.in_names
        ]

    def _zeros(self):
        return [np.zeros((M * s[0], *s[1:]), d) for s, d in self.zero_shapes]

    def _split(self, out_arrs):
        res = []
        for c in range(M):
            res.append(
                {
                    name: np.asarray(out_arrs[i]).reshape(M, *self.out_avals[i].shape)[c]
                    for i, name in enumerate(self.out_names)
                }
            )
        return res

    def run(self, in_maps):
        out_arrs = self.fn(*self._concat_inputs(in_maps), *self._zeros())
        return self._split(out_arrs)

    def bench(self, in_maps, iters=20):
        import time

        jax = self.jax
        dev_in = [
            jax.device_put(a, self.sharding) for a in self._concat_inputs(in_maps)
        ]
        times = []
        out_arrs = None
        for _ in range(iters):
            zeros = [jax.device_put(z, self.sharding) for z in self._zeros()]
            jax.block_until_ready(zeros)
            t0 = time.perf_counter()
            out_arrs = self.fn(*dev_in, *zeros)
            jax.block_until_ready(out_arrs)
            times.append(time.perf_counter() - t0)
        return self._split(out_arrs), times


_runner_cache = {}


def _get_runner(nc, key):
    if key not in _runner_cache:
        _runner_cache[key] = _Runner(nc)
    return _runner_cache[key]


def _run_sim(nc, in_maps):
    from concourse.bass_interp import CoreSim

    outs = []
    for im in in_maps:
        sim = CoreSim(nc)
        for k, v in im.items():
            sim.tensor(k)[:] = v
        sim.simulate()
        out_name = "pt" if "xw" in im else "ot"
        outs.append({out_name: np.array(sim.tensor(out_name))})
    return outs


def _run(inputs, mode=None, trace=False, backend="hw", bench_iters=0):
    nc1, nc2 = _get_programs(mode)
    info = {}

    in_maps1 = _prep_phase1_inputs(inputs, mode)
    if backend == "sim":
        res1 = _run_sim(nc1, in_maps1)
    else:
        runner1 = _get_runner(nc1, ("p1",))
        if bench_iters:
            res1, times = runner1.bench(in_maps1, bench_iters)
            info["phase1_ns"] = int(min(times) * 1e9)
            info["phase1_mean_ns"] = float(np.mean(times) * 1e9)
        else:
            res1 = runner1.run(in_maps1)
    pts = [
        np.asarray(res1[c]["pt"], np.float16).transpose(1, 0, 2).reshape(AV, B)
        for c in range(M)
    ]

    in_maps2 = _prep_phase2_inputs(inputs, pts)
    if backend == "sim":
        res2 = _run_sim(nc2, in_maps2)
    else:
        runner2 = _get_runner(nc2, ("p2",))
        if bench_iters:
            res2, times = runner2.bench(in_maps2, bench_iters)
            info["phase2_ns"] = int(min(times) * 1e9)
            info["phase2_mean_ns"] = float(np.mean(times) * 1e9)
        else:
            res2 = runner2.run(in_maps2)

    out = np.empty((B, NDIV, PER), np.float32)
    for c in range(M):
        ot = np.asarray(res2[c]["ot"], np.float32)  # [128, 2, BT]
        full = np.concatenate([ot[:, 0, :], ot[0:64, 1, :]], axis=0)
        out[BT * c : BT * (c + 1)] = full.T.reshape(BT, NDIV, PER)
    return out, info


def _p25(ts):
    ts = sorted(ts)
    return ts[max(0, len(ts) // 4)]


def bench_hw(inputs, mode=None, big_rep=9, iters=12):
    """Differential HW timing: (T(R) - T(1)) / (R - 1) cancels the axon
    launch overhead and measures the true per-pass device time.  Uses the
    25th percentile (the min is occasionally glitchy on the relay)."""
    in_maps1 = _prep_phase1_inputs(inputs, mode)
    res = {}
    est = {}
    for r in (1, big_rep):
        nc1, _ = _get_programs(mode, r)
        runner = _get_runner(nc1, ("p1", r))
        out1, times = runner.bench(in_maps1, iters)
        est[r] = _p25(times)
    res["phase1_ns"] = (est[big_rep] - est[1]) / (big_rep - 1) * 1e9
    res["phase1_launch_ns"] = est[1] * 1e9

    pts = [
        np.asarray(o["pt"], np.float16).transpose(1, 0, 2).reshape(AV, B)
        for o in out1
    ]
    in_maps2 = _prep_phase2_inputs(inputs, pts)
    for r in (1, big_rep):
        _, nc2 = _get_programs(mode, r)
        runner = _get_runner(nc2, ("p2", r))
        _, times = runner.bench(in_maps2, iters)
        est[r] = _p25(times)
    res["phase2_ns"] = (est[big_rep] - est[1]) / (big_rep - 1) * 1e9
    res["phase2_launch_ns"] = est[1] * 1e9
    return res


def kernel(**inputs):
    out, _ = _run(inputs)
    return out


# revision 67
# speedup vs baseline: 1.0036x; 1.0036x over previous
"""Trainium2 Bass kernel for nn_BHSDuelingDQN (gnn_message_passing).

Math notes (validated vs reference to fp32 precision):
  - The edge MLP input is ones(E,1), so every edge shares one theta [F,OUT]:
        theta = (relu(w1[0]+b1) @ w2 + b2).reshape(F, OUT)
  - edge_index values live in [0, N), so the gather/scatter-add only touches
    batch 0 of flat=[B*N,F].  With C[s,t] = #edges(src=s, tgt=t):
        agg(batch0) = C^T @ (x[0] @ theta)
    which turns the whole message passing into dense matmuls.  C is built on
    the host from edge_index (pure index bookkeeping; all FLOPs with x /
    theta / weights stay on device).

Sharding: phase 1 is node-sharded (each of 8 cores owns 32 of 256 nodes and
computes partial pre-activations of adv/v1 for all 1024 batches over its
4096 feature rows).  Phase 2 is batch-sharded (each core sums the 8 partials
for its 128 batches and runs the small value-head + dueling combine).  The
host only slices / concatenates / transposes arrays between phases.

Phase-1 schedule: the PE is the bottleneck (conv 32768 + accumulate 65536
cycles per core at full f32r rate).  Everything else is built around keeping
the PE gapless and fully ramped:
  - warm-up matmuls on memset/identity data bridge the initial DMA latency
    AND the p-state ramp (PE runs 2x slower for its first ~3us of busy time).
  - one flat conv/acc pipeline across three batch sweeps (512/256/256, the
    last narrow so the final drain+DMA tail is short); convs run 3 nodes
    ahead of the accumulates so the relu (alternating Act/DVE; GPSIMD cannot
    read PSUM) is never on the PE critical path.
  - head biases are folded into the drains; the batch-0 column (the only one
    with a scatter-add contribution) is patched into each feat tile from a
    precomputed feat0 column.  Phase-1 partials go to HBM as fp16.
Phase 2 folds the dueling mean/expand structure into the weights on the host
(pure index structure), so each 64-batch chunk is 3 fp16 tree-adds + relu +
10 small matmuls; the 8-partial reduction and all model FLOPs stay on device.

HW-ISA notes learned the hard way: GPSIMD cannot access PSUM; fp32r matmuls
need moving-dim >= 256 for full rate and reject 1-column outputs; matmul
fmap/weight must share their SBUF start partition, which must equal the PE
row tile position; f32->f32r bitcasts of engine outputs are rejected (only
engines and DMA may produce f32r); f32r/f16 memsets are invalid ISA.
"""

import os
from contextlib import ExitStack

import numpy as np

import concourse.bacc as bacc
import concourse.bass as bass
import concourse.mybir as mybir
import concourse.tile as tile
from concourse import masks
from concourse.bass_utils import run_bass_kernel_spmd  # noqa: F401  (contract)

F32 = mybir.dt.float32
F32R = mybir.dt.float32r
F16 = mybir.dt.float16
BF16 = mybir.dt.bfloat16

B, N, F, E, OUT, NDIV, PER = 1024, 256, 8, 1024, 128, 64, 3
NADV = NDIV * PER            # 192
AV = NADV + 64               # 256 fused output cols of phase 1 (adv | v1)
M = 8                        # cores
NPC = N // M                 # 32 nodes per core
MODE = os.environ.get("BASS_KERNEL_MODE", "f32r")  # kept for test.py compat

WARM_N = int(os.environ.get("BASS_WARM_N", "6"))

# rwx column map (f32r, first DMA on the sync queue: everything the sweep's
# start needs — root weight replicas, conv bias, head-bias columns, edge MLP)
RX_RW = 0                     # 128 cols: root_w at partitions 32j..32j+8
RX_CB, RX_BR0, RX_BR1, RX_W1T, RX_B1T = 128, 129, 130, 131, 132
RX_X0TL = 133                 # 32 cols [8, 32]: x[0] local nodes, transposed
RX_COLS = 165

# pa0 column map (f32, scalar queue, needed only by the phase-0 chain)
PA_B2T = 0                    # 8 cols  [128, 8] = b2.reshape(F, OUT).T
PA_X0T = 8                    # 256 cols [8, 256] = x[0].T
PA_C = 264                    # 64 cols  [128, 2, 32] edge-count matrix
PA0_COLS = PA_C + 2 * NPC     # 328

# batch sweeps: (xw column base, width, output batch base); the last one is
# narrow so the final drain + output DMA tail is short.  Batch 0 (the only
# one with a scatter-add contribution) sits at sweep 1's first column.
SWEEPS = ((0, 512, 512), (512, 256, 0), (768, 256, 256))

_build_cache = {}


def _build_phase1(repeat=1):
    nc = bacc.Bacc("TRN2")

    pa0_d = nc.dram_tensor("pa0", [128, PA0_COLS], F32, kind="ExternalInput")
    pw2_d = nc.dram_tensor("pw2", [64, F * OUT], F32, kind="ExternalInput")
    # rwx rides at the head of xw so the critical first transfer is ONE DMA
    # (HWDGE descriptor generation is a single shared ~625ns/DMA resource)
    xw_d = nc.dram_tensor(
        "xw", [128, RX_COLS + 8 * 1024], F32R, kind="ExternalInput"
    )
    wb_d = nc.dram_tensor("wb", [128, 8, 1024], BF16, kind="ExternalInput")
    pt_d = nc.dram_tensor("pt", [128, 2, B], F16, kind="ExternalOutput")

    with tile.TileContext(nc) as tc:
      for rep in range(repeat):
        with ExitStack() as ctx:
            const = ctx.enter_context(tc.tile_pool(name=f"const{rep}", bufs=1))

            # tiny device-generated tensors first: no DMA deps
            wsmall_sb = const.tile([128, 16], F32, name="wsmall_sb")
            nc.gpsimd.memset(wsmall_sb, 0.0)
            ident_sb = const.tile([128, 128], F32, name="ident_sb")
            masks.make_identity(nc, ident_sb)

            # DMAs in consumption order
            xw_sb = const.tile([128, RX_COLS + 8 * 1024], F32R, name="xw_sb")
            wb_sb = const.tile([128, 8, 1024], BF16, name="wb_sb")
            rwx_sb = xw_sb[:, 0:RX_COLS]

            def xcol(g, base):
                return RX_COLS + g * 1024 + base

            nc.sync.dma_start(
                out=xw_sb[:, 0 : xcol(0, 512)], in_=xw_d[:, 0 : xcol(0, 512)]
            )
            nc.sync.dma_start(out=wb_sb[:, 0, 0:512], in_=wb_d[:, 0, 0:512])
            nc.sync.dma_start(out=wb_sb[:, 0, 512:1024], in_=wb_d[:, 0, 512:1024])
            pa0_sb = const.tile([128, PA0_COLS], F32, name="pa0_sb")
            nc.scalar.dma_start(out=pa0_sb, in_=pa0_d[:])
            pw2_sb = const.tile([64, F * OUT], F32, name="pw2_sb")
            nc.scalar.dma_start(out=pw2_sb, in_=pw2_d[:])
            for g in range(1, 8):
                nc.sync.dma_start(
                    out=xw_sb[:, xcol(g, 0) : xcol(g, 512)],
                    in_=xw_d[:, xcol(g, 0) : xcol(g, 512)],
                )
                nc.sync.dma_start(out=wb_sb[:, g], in_=wb_d[:, g])
            for g in range(8):
                nc.sync.dma_start(
                    out=xw_sb[:, xcol(g, 512) : xcol(g, 1024)],
                    in_=xw_d[:, xcol(g, 512) : xcol(g, 1024)],
                )

            def pa(col, ncols, nrows=128):
                return pa0_sb[0:nrows, col : col + ncols]

            def rx(col, ncols, nrows=128):
                # f32 bitcast view (f32r is only needed for matmul operands)
                return rwx_sb[0:nrows, col : col + ncols].bitcast(F32)

            cb_ap = rx(RX_CB, 1)

            # small SBUF intermediates
            h_sb = const.tile([64, 1], F32, name="h_sb")
            thT_sb = const.tile([128, F], F32, name="thT_sb")
            th_sb = const.tile([F, OUT], F32, name="th_sb")
            x0th_sb = const.tile([128, 2, OUT], F32, name="x0th_sb")
            feat0_sb = const.tile([128, NPC], BF16, name="feat0_sb")

            acc_pool = ctx.enter_context(
                tc.tile_pool(name=f"accp{rep}", bufs=2, space="PSUM")
            )
            conv_pool = ctx.enter_context(
                tc.tile_pool(name=f"convp{rep}", bufs=3, space="PSUM")
            )
            p0_pool = ctx.enter_context(
                tc.tile_pool(name=f"p0p{rep}", bufs=1, space="PSUM")
            )
            feat_pool = ctx.enter_context(tc.tile_pool(name=f"featp{rep}", bufs=6))
            out_pool = ctx.enter_context(tc.tile_pool(name=f"outp{rep}", bufs=1))
            out_sb = out_pool.tile([128, 2, B], F16, name="out_sb")

            # ---- PE warm-up: no-dep fp32 matmuls (4 cycles/row) bridge the
            # initial DMA latency and the p-state ramp; the first few are
            # narrow so the PE starts the moment the 16-col memset lands
            for k in range(5):
                warm_ps = conv_pool.tile(
                    [16, 16], F32, name="warm_ps", tag="conv_ps"
                )
                nc.tensor.matmul(warm_ps, wsmall_sb[:, 0:16], wsmall_sb)
            for k in range(WARM_N):
                warm_ps = conv_pool.tile(
                    [128, 128], F32, name="warm_ps", tag="conv_ps"
                )
                nc.tensor.matmul(warm_ps, ident_sb, ident_sb)

            def wc_ap(n, m):
                base = (n % 4) * 256 + m * 128
                return wb_sb[:, n // 4, base : base + 128]

            RELU_ENG = ("act", "dve")

            def emit_conv(n, xbase, width, agg):
                j = n % 4
                conv_ps = conv_pool.tile(
                    [128, width], F32, name="conv_ps", tag="conv_ps"
                )
                nc.tensor.matmul(
                    conv_ps,
                    rwx_sb[32 * j : 32 * j + F, RX_RW : RX_RW + OUT],
                    xw_sb[
                        32 * j : 32 * j + F,
                        xcol(n // 4, xbase) : xcol(n // 4, xbase) + width,
                    ],
                    tile_position=(32 * j, 0),
                )
                feat_sb = feat_pool.tile([128, width], BF16, name="feat_sb")
                eng = RELU_ENG[n % 2]
                if eng == "act":
                    nc.scalar.activation(
                        feat_sb,
                        conv_ps,
                        mybir.ActivationFunctionType.Relu,
                        bias=cb_ap,
                    )
                elif eng == "dve":
                    nc.vector.tensor_scalar(
                        feat_sb, conv_ps, cb_ap, 0.0,
                        mybir.AluOpType.add, mybir.AluOpType.max,
                    )
                if agg:
                    # batch 0 (column 0) gets the precomputed scatter-add
                    # corrected feature column for this node
                    if n % 2 == 0:
                        nc.vector.tensor_copy(
                            feat_sb[:, 0:1], feat0_sb[:, n : n + 1]
                        )
                    else:
                        nc.scalar.activation(
                            feat_sb[:, 0:1],
                            feat0_sb[:, n : n + 1],
                            mybir.ActivationFunctionType.Copy,
                        )
                return feat_sb

            acc_tiles = {}

            def drain(si):
                xbase, width, bbase = SWEEPS[si]
                sl = slice(bbase, bbase + width)
                acc_ps = acc_tiles[si]
                nc.vector.tensor_scalar_add(
                    out_sb[:, 0, sl], acc_ps[0], rx(RX_BR0, 1)
                )
                nc.scalar.activation(
                    out_sb[:, 1, sl],
                    acc_ps[1],
                    mybir.ActivationFunctionType.Identity,
                    bias=rx(RX_BR1, 1),
                )
                nc.sync.dma_start(out=pt_d[:, :, sl], in_=out_sb[:, :, sl])

            def run_sweeps(inject=()):
                # one flat conv/acc pipeline across all sweeps: convs run a
                # few nodes ahead so sweep boundaries leave no PE gap
                inject = dict(inject)
                pend = []

                def emit_acc(si, n, feat_sb):
                    for m in range(2):
                        nc.tensor.matmul(
                            acc_tiles[si][m],
                            wc_ap(n, m),
                            feat_sb,
                            start=(n == 0),
                            stop=(n == NPC - 1),
                        )
                    if n == NPC - 1:
                        drain(si)

                events = [(si, n) for si in range(len(SWEEPS))
                          for n in range(NPC)]
                for gi, (si, n) in enumerate(events):
                    if n == 0:
                        acc_tiles[si] = [
                            acc_pool.tile(
                                [128, SWEEPS[si][1]], F32,
                                name=f"acc{m}_{si}", tag=f"acc{m}",
                            )
                            for m in range(2)
                        ]
                    pend.append(
                        (si, n, emit_conv(n, SWEEPS[si][0], SWEEPS[si][1],
                                          si == 1))
                    )
                    if gi in inject:
                        inject[gi]()
                    if len(pend) > 3:
                        psi, pn, pf = pend.pop(0)
                        emit_acc(psi, pn, pf)
                for psi, pn, pf in pend:
                    emit_acc(psi, pn, pf)

            # ---- phase 0 steps: theta, x0@theta, aggT -------------------
            # interleaved into the h=1 sweep so the serial chain (with its
            # cross-engine semaphore latencies) hides behind conv/acc work
            def p0_theta():
                nc.vector.tensor_scalar(
                    h_sb, rx(RX_W1T, 1, 64), rx(RX_B1T, 1, 64), 0.0,
                    mybir.AluOpType.add, mybir.AluOpType.max,
                )
                thT_ps = p0_pool.tile([128, F], F32, name="thT_ps", tag="p0")
                for f in range(F):
                    nc.tensor.matmul(
                        thT_ps[:, f : f + 1],
                        pw2_sb[:, f * OUT : (f + 1) * OUT],
                        h_sb,
                    )
                nc.vector.tensor_add(thT_sb, thT_ps, pa(PA_B2T, F))

            def p0_th():
                th_ps = p0_pool.tile([F, OUT], F32, name="th_ps", tag="p0")
                nc.tensor.transpose(th_ps, thT_sb[:, 0:F], ident_sb)
                nc.vector.tensor_copy(th_sb, th_ps)

            def p0_x0th(s):
                x0th_ps = p0_pool.tile(
                    [128, OUT], F32, name=f"x0th_ps{s}", tag="p0"
                )
                nc.tensor.matmul(
                    x0th_ps, pa(PA_X0T + s * 128, 128, F), th_sb
                )
                nc.vector.tensor_copy(x0th_sb[:, s, :], x0th_ps)

            def p0_feat0():
                agg_ps = p0_pool.tile([128, NPC], F32, name="agg_ps", tag="p0")
                for s in range(2):
                    nc.tensor.matmul(
                        agg_ps,
                        x0th_sb[:, s, :],
                        pa(PA_C + s * NPC, NPC),
                        start=(s == 0),
                        stop=False,
                    )
                nc.tensor.matmul(
                    agg_ps,
                    rwx_sb[0:F, RX_RW : RX_RW + OUT],
                    rwx_sb[0:F, RX_X0TL : RX_X0TL + NPC],
                    start=False,
                    stop=True,
                )
                nc.scalar.activation(
                    feat0_sb,
                    agg_ps,
                    mybir.ActivationFunctionType.Relu,
                    bias=cb_ap,
                )

            # sweep 0 carries the phase-0 chain; sweep 1 needs aggT ready
            run_sweeps(inject={
                8: p0_theta,
                11: p0_th,
                14: lambda: p0_x0th(0),
                16: lambda: p0_x0th(1),
                18: p0_feat0,
            })

    nc.finalize()
    return nc


# phase-2 folded-weight layout (all structural matrices pre-multiplied into
# the weights on the host; the dueling mean/expand matrices are pure index
# structure so this is weight re-indexing, not model compute)
PF_V2W = 0      # [65, 64]  v2w with v2b as K row 64
PF_A0 = 64      # [128, 128] I + MGA@EM0
PF_B0 = 192     # [64, 128]  MGB@EM0
PF_C0 = 320     # [64, 128]  V3@EM0
PF_CB0 = 448    # [1, 128]   v3b@EM0
PF_A1 = 576     # [128, 64]  MGA@EM1
PF_B1 = 640     # [64, 64]   I + MGB@EM1
PF_C1 = 704     # [64, 64]   V3@EM1
PF_CB1 = 768    # [1, 64]    v3b@EM1
PF_COLS = 832
BT = B // M      # 128 batches per core
# even batch chunks, each a contiguous dram tensor (strided chunk slices of
# one flat tensor fall under the 512B inner-run threshold and pay a 2x DMA
# latency multiplier)
P2_CHUNKS = ((0, 64), (64, 64))


def _build_phase2(repeat=1):
    nc = bacc.Bacc("TRN2")

    parts0_d = nc.dram_tensor("parts0", [128, 3, M, 64], F16, kind="ExternalInput")
    parts1_d = nc.dram_tensor("parts1", [128, 3, M, 64], F16, kind="ExternalInput")
    pf_d = nc.dram_tensor("pf", [128, PF_COLS], F16, kind="ExternalInput")
    ot_d = nc.dram_tensor("ot", [128, 2, BT], F32, kind="ExternalOutput")

    with tile.TileContext(nc) as tc:
      for rep in range(repeat):
        with ExitStack() as ctx:
            const = ctx.enter_context(tc.tile_pool(name=f"c2_{rep}", bufs=1))

            ones_sb = const.tile([128, BT], F16, name="ones_sb")
            nc.gpsimd.memset(ones_sb, 1.0)

            pp0_sb = const.tile([128, 3, M, 64], F16, name="pp0_sb")
            pp1_sb = const.tile([128, 3, M, 64], F16, name="pp1_sb")
            pf_sb = const.tile([128, PF_COLS], F16, name="pf_sb")
            nc.sync.dma_start(out=pp0_sb, in_=parts0_d[:])
            nc.scalar.dma_start(out=pf_sb, in_=pf_d[:])
            nc.sync.dma_start(out=pp1_sb, in_=parts1_d[:])

            def pf(col, ncols, nrows=128):
                return pf_sb[0:nrows, col : col + ncols]

            work = ctx.enter_context(tc.tile_pool(name=f"work{rep}", bufs=2))
            psum = ctx.enter_context(
                tc.tile_pool(name=f"psum{rep}", bufs=2, space="PSUM")
            )
            ot_sb = const.tile([128, 2, BT], F32, name="ot_sb")
            nc.gpsimd.memset(ot_sb[64:128, 1, :], 0.0)

            for (b0, bw), pp_t in zip(P2_CHUNKS, (pp0_sb, pp1_sb)):
                pp = pp_t[:]
                bsl = slice(b0, b0 + bw)
                eng = nc.vector
                # sum the 8 partials (tree on one engine: no sem hops;
                # biases already folded in by phase 1), then one relu
                s1 = work.tile([128, 3, 4, bw], F16, name="s1")
                eng.tensor_add(s1, pp[:, :, 0:4, :], pp[:, :, 4:8, :])
                s2 = work.tile([128, 3, 2, bw], F16, name="s2")
                eng.tensor_add(s2, s1[:, :, 0:2, :], s1[:, :, 2:4, :])
                r_all = work.tile([128, 3, bw], F16, name="r_all")
                s3 = work.tile([128, 3, bw], F16, name="s3")
                eng.tensor_add(s3, s2[:, :, 0, :], s2[:, :, 1, :])
                eng.tensor_scalar_max(r_all, s3, 0.0)
                ar0 = r_all[:, 0, :]
                ar1 = r_all[0:64, 1, :]
                v1r = r_all[0:64, 2, :]

                # v2 = relu(v1 @ v2w + v2b)
                v2_ps = psum.tile([64, bw], F32, name="v2_ps")
                nc.tensor.matmul(
                    v2_ps, pf(PF_V2W, 64, 64), v1r, start=True, stop=False
                )
                nc.tensor.matmul(
                    v2_ps, pf_sb[64:65, PF_V2W : PF_V2W + 64], ones_sb[64:65, 0:bw],
                    start=False, stop=True, tile_position=(64, 0),
                )
                v2r = work.tile([64, bw], F16, name="v2r")
                eng.tensor_scalar_max(v2r, v2_ps, 0.0)

                # folded dueling combine: out0 = A0^T ar0 + B0^T ar1
                #                              + C0^T v2r + cb0^T ones
                o0_ps = psum.tile([128, bw], F32, name="o0_ps")
                nc.tensor.matmul(o0_ps, pf(PF_A0, 128), ar0, start=True, stop=False)
                nc.tensor.matmul(o0_ps, pf(PF_B0, 128, 64), ar1, start=False, stop=False)
                nc.tensor.matmul(o0_ps, pf(PF_C0, 128, 64), v2r, start=False, stop=False)
                nc.tensor.matmul(
                    o0_ps, pf(PF_CB0, 128, 1), ones_sb[0:1, 0:bw],
                    start=False, stop=True, tile_position=(0, 0),
                )
                o1_ps = psum.tile([64, bw], F32, name="o1_ps")
                nc.tensor.matmul(o1_ps, pf(PF_A1, 64), ar0, start=True, stop=False)
                nc.tensor.matmul(o1_ps, pf(PF_B1, 64, 64), ar1, start=False, stop=False)
                nc.tensor.matmul(o1_ps, pf(PF_C1, 64, 64), v2r, start=False, stop=False)
                nc.tensor.matmul(
                    o1_ps, pf(PF_CB1, 64, 1), ones_sb[0:1, 0:bw],
                    start=False, stop=True, tile_position=(0, 0),
                )
                nc.scalar.activation(
                    ot_sb[:, 0, bsl], o0_ps, mybir.ActivationFunctionType.Copy
                )
                eng.tensor_copy(ot_sb[0:64, 1, bsl], o1_ps)
                nc.sync.dma_start(out=ot_d[:, :, bsl], in_=ot_sb[:, :, bsl])

    nc.finalize()
    return nc


def _get_programs(mode=None, repeat=1):
    key = repeat
    if key not in _build_cache:
        _build_cache[key] = (_build_phase1(repeat), _build_phase2(repeat))
    return _build_cache[key]


def _prep_phase1_inputs(inputs, mode=None):
    x = np.ascontiguousarray(np.asarray(inputs["x"], np.float32))
    ei = np.asarray(inputs["edge_index"]).astype(np.int64)
    w1 = np.asarray(inputs["w1"], np.float32)
    b1 = np.asarray(inputs["b1"], np.float32)
    w2 = np.asarray(inputs["w2"], np.float32)
    b2 = np.asarray(inputs["b2"], np.float32)
    root_w = np.asarray(inputs["root_w"], np.float32)
    conv_b = np.asarray(inputs["conv_b"], np.float32)
    adv_w = np.asarray(inputs["adv_w"], np.float32)
    v1w = np.asarray(inputs["v1w"], np.float32)
    adv_b = np.asarray(inputs["adv_b"], np.float32)
    v1b = np.asarray(inputs["v1b"], np.float32)

    src_i, tgt_i = ei[0], ei[1]
    wfull = np.concatenate([adv_w, v1w], axis=1)  # [32768, 256]

    rwx = np.zeros((128, RX_COLS), np.float32)
    for j in range(4):
        rwx[32 * j : 32 * j + F, RX_RW : RX_RW + OUT] = root_w
    rwx[:, RX_CB] = conv_b
    rwx[0:64, RX_W1T] = w1.reshape(64)
    rwx[0:64, RX_B1T] = b1

    pa0 = np.zeros((128, PA0_COLS), np.float32)
    pa0[:, PA_B2T : PA_B2T + F] = b2.reshape(F, OUT).T
    pa0[0:F, PA_X0T : PA_X0T + N] = x[0].T

    brow = np.concatenate([adv_b, v1b])          # [256]

    in_maps = []
    for c in range(M):
        rwc = rwx.copy()
        if c == 0:
            rwc[:, RX_BR0] = brow[0:128]
            rwc[:, RX_BR1] = brow[128:256]
        rwc[0:F, RX_X0TL : RX_X0TL + NPC] = x[0, NPC * c : NPC * (c + 1), :].T
        pac = pa0.copy()
        # edge-count matrix for this core's 32 target nodes
        cmat = np.zeros((N, NPC), np.float32)
        sel = (tgt_i >= NPC * c) & (tgt_i < NPC * (c + 1))
        np.add.at(cmat, (src_i[sel], tgt_i[sel] - NPC * c), 1.0)
        pac[:, PA_C : PA_C + NPC] = cmat[0:128]
        pac[:, PA_C + NPC : PA_C + 2 * NPC] = cmat[128:256]

        xw = np.zeros((128, 8, 1024), np.float32)
        xc = x[:, NPC * c : NPC * (c + 1), :]           # [B, 32, 8]
        xr = xc.transpose(1, 2, 0)                      # [32, 8, B]
        for j in range(4):
            # node 4g+j sits at partitions 32j..32j+F of group g
            xw[32 * j : 32 * j + F, :, 0:512] = xr[j::4, :, 512:1024].transpose(
                1, 0, 2
            )
            xw[32 * j : 32 * j + F, :, 512:1024] = xr[j::4, :, 0:512].transpose(
                1, 0, 2
            )
        import ml_dtypes
        rows = wfull[4096 * c : 4096 * (c + 1)]         # [4096, 256]
        wb = (
            rows.reshape(8, 4, 128, AV)
            .transpose(2, 0, 1, 3)
            .reshape(128, 8, 1024)
        ).astype(ml_dtypes.bfloat16)
        xwf = np.concatenate([rwc, xw.reshape(128, 8 * 1024)], axis=1)
        in_maps.append(
            {"pa0": pac, "pw2": w2, "xw": xwf, "wb": wb}
        )
    return in_maps


def _prep_phase2_inputs(inputs, pts):
    v2w = np.asarray(inputs["v2w"], np.float32)
    v2b = np.asarray(inputs["v2b"], np.float32)
    v3w = np.asarray(inputs["v3w"], np.float32)
    v3b = np.asarray(inputs["v3b"], np.float32)

    # structural dueling matrices (index structure only)
    dp = np.arange(NADV)
    mg = np.zeros((NADV, NDIV), np.float32)
    mg[dp, dp // PER] = -1.0 / PER           # negated group-mean matrix
    em = np.zeros((NDIV, NADV), np.float32)  # expand d -> (d,p)
    em[dp // PER, dp] = 1.0
    em0, em1 = em[:, :128], em[:, 128:]
    mga, mgb = mg[:128], mg[128:]

    pfm = np.zeros((128, PF_COLS), np.float32)
    pfm[0:64, PF_V2W : PF_V2W + 64] = v2w
    pfm[64, PF_V2W : PF_V2W + 64] = v2b
    pfm[:, PF_A0 : PF_A0 + 128] = np.eye(128, dtype=np.float32) + mga @ em0
    pfm[0:64, PF_B0 : PF_B0 + 128] = mgb @ em0
    pfm[0:64, PF_C0 : PF_C0 + 128] = v3w @ em0
    pfm[0, PF_CB0 : PF_CB0 + 128] = v3b @ em0
    pfm[:, PF_A1 : PF_A1 + 64] = mga @ em1
    pfm[0:64, PF_B1 : PF_B1 + 64] = np.eye(64, dtype=np.float32) + mgb @ em1
    pfm[0:64, PF_C1 : PF_C1 + 64] = v3w @ em1
    pfm[0, PF_CB1 : PF_CB1 + 64] = v3b @ em1
    pfm = pfm.astype(np.float16)

    in_maps = []
    for c in range(M):
        bsl = slice(BT * c, BT * (c + 1))
        stk = np.stack([p[:, bsl] for p in pts])              # [i, 256, BT]
        parts = np.zeros((128, 3, M, BT), np.float16)
        parts[:, 0] = stk[:, 0:128, :].transpose(1, 0, 2)
        parts[0:64, 1] = stk[:, 128:NADV, :].transpose(1, 0, 2)
        parts[0:64, 2] = stk[:, NADV:AV, :].transpose(1, 0, 2)
        in_maps.append({
            "parts0": np.ascontiguousarray(parts[:, :, :, 0:64]),
            "parts1": np.ascontiguousarray(parts[:, :, :, 64:128]),
            "pf": pfm,
        })
    return in_maps


class _Runner:
    """Cached PJRT executor for one Bass program across the 8 cores.

    Mirrors bass2jax.run_bass_via_pjrt but keeps the jitted callable so
    repeat calls don't re-trace/re-lower, enabling benchmarking.
    """

    def __init__(self, nc):
        import jax
        from jax.sharding import Mesh, PartitionSpec, NamedSharding
        from jax.experimental.shard_map import shard_map
        from concourse import bass2jax

        bass2jax.install_neuronx_cc_hook()
        self.jax = jax
        self.nc = nc
        partition_name = (
            nc.partition_id_tensor.name if nc.partition_id_tensor else None
        )
        in_names, out_names, out_avals, zero_shapes = [], [], [], []
        for alloc in nc.m.functions[0].allocations:
            if not isinstance(alloc, mybir.MemoryLocationSet):
                continue
            name = alloc.memorylocations[0].name
            if alloc.kind == "ExternalInput":
                if name != partition_name:
                    in_names.append(name)
            elif alloc.kind == "ExternalOutput":
                shape = tuple(alloc.tensor_shape)
                dtype = mybir.dt.np(alloc.dtype)
                out_names.append(name)
                out_avals.append(jax.core.ShapedArray(shape, dtype))
                zero_shapes.append((shape, dtype))
        self.in_names, self.out_names = in_names, out_names
        self.out_avals, self.zero_shapes = out_avals, zero_shapes
        n_params, n_outs = len(in_names), len(out_names)
        self.n_params = n_params

        bind_names = in_names + out_names
        if partition_name is not None:
            bind_names = bind_names + [partition_name]

        def _body(*args):
            operands = list(args)
            if partition_name is not None:
                operands.append(bass2jax.partition_id_tensor())
            outs = bass2jax._bass_exec_p.bind(
                *operands,
                out_avals=tuple(out_avals),
                in_names=tuple(bind_names),
                out_names=tuple(out_names),
                lowering_input_output_aliases=(),
                sim_require_finite=True,
                sim_require_nnan=True,
                nc=nc,
            )
            return tuple(outs)

        devices = jax.devices()[:M]
        self.mesh = Mesh(np.asarray(devices), ("core",))
        spec = PartitionSpec("core")
        self.sharding = NamedSharding(self.mesh, spec)
        donate = tuple(range(n_params, n_params + n_outs))
        self.fn = jax.jit(
            shard_map(
                _body,
                mesh=self.mesh,
                in_specs=(spec,) * (n_params + n_outs),
                out_specs=(spec,) * n_outs,
                check_rep=False,
            ),
            donate_argnums=donate,
            keep_unused=True,
        )

    def _concat_inputs(self, in_maps):
        return [
            np.concatenate([np.asarray(m[name]) for m in in_maps], axis=0)
            for name in self.in_names
        ]

    def _zeros(self):
        return [np.zeros((M * s[0], *s[1:]), d) for s, d in self.zero_shapes]

    def _split(self, out_arrs):
        res = []
        for c in range(M):
            res.append(
                {
                    name: np.asarray(out_arrs[i]).reshape(M, *self.out_avals[i].shape)[c]
                    for i, name in enumerate(self.out_names)
                }
            )
        return res

    def run(self, in_maps):
        out_arrs = self.fn(*self._concat_inputs(in_maps), *self._zeros())
        return self._split(out_arrs)

    def bench(self, in_maps, iters=20):
        import time

        jax = self.jax
        dev_in = [
            jax.device_put(a, self.sharding) for a in self._concat_inputs(in_maps)
        ]
        times = []
        out_arrs = None
        for _ in range(iters):
            zeros = [jax.device_put(z, self.sharding) for z in self._zeros()]
            jax.block_until_ready(zeros)
            t0 = time.perf_counter()
            out_arrs = self.fn(*dev_in, *zeros)
            jax.block_until_ready(out_arrs)
            times.append(time.perf_counter() - t0)
        return self._split(out_arrs), times


_runner_cache = {}


def _get_runner(nc, key):
    if key not in _runner_cache:
        _runner_cache[key] = _Runner(nc)
    return _runner_cache[key]


def _run_sim(nc, in_maps):
    from concourse.bass_interp import CoreSim

    outs = []
    for im in in_maps:
        sim = CoreSim(nc)
        for k, v in im.items():
            sim.tensor(k)[:] = v
        sim.simulate()
        out_name = "pt" if "xw" in im else "ot"
        outs.append({out_name: np.array(sim.tensor(out_name))})
    return outs


def _run(inputs, mode=None, trace=False, backend="hw", bench_iters=0):
    nc1, nc2 = _get_programs(mode)
    info = {}

    in_maps1 = _prep_phase1_inputs(inputs, mode)
    if backend == "sim":
        res1 = _run_sim(nc1, in_maps1)
    else:
        runner1 = _get_runner(nc1, ("p1",))
        if bench_iters:
            res1, times = runner1.bench(in_maps1, bench_iters)
            info["phase1_ns"] = int(min(times) * 1e9)
            info["phase1_mean_ns"] = float(np.mean(times) * 1e9)
        else:
            res1 = runner1.run(in_maps1)
    pts = [
        np.asarray(res1[c]["pt"], np.float16).transpose(1, 0, 2).reshape(AV, B)
        for c in range(M)
    ]

    in_maps2 = _prep_phase2_inputs(inputs, pts)
    if backend == "sim":
        res2 = _run_sim(nc2, in_maps2)
    else:
        runner2 = _get_runner(nc2, ("p2",))
        if bench_iters:
            res2, times = runner2.bench(in_maps2, bench_iters)
            info["phase2_ns"] = int(min(times) * 1e9)
            info["phase2_mean_ns"] = float(np.mean(times) * 1e9)
        else:
            res2 = runner2.run(in_maps2)

    out = np.empty((B, NDIV, PER), np.float32)
    for c in range(M):
        ot = np.asarray(res2[c]["ot"], np.float32)  # [128, 2, BT]
        full = np.concatenate([ot[:, 0, :], ot[0:64, 1, :]], axis=0)
        out[BT * c : BT * (c + 1)] = full.T.reshape(BT, NDIV, PER)
    return out, info


def _p25(ts):
    ts = sorted(ts)
    return ts[max(0, len(ts) // 4)]


def bench_hw(inputs, mode=None, big_rep=9, iters=12):
    """Differential HW timing: (T(R) - T(1)) / (R - 1) cancels the axon
    launch overhead and measures the true per-pass device time.  Uses the
    25th percentile (the min is occasionally glitchy on the relay)."""
    in_maps1 = _prep_phase1_inputs(inputs, mode)
    res = {}
    est = {}
    for r in (1, big_rep):
        nc1, _ = _get_programs(mode, r)
        runner = _get_runner(nc1, ("p1", r))
        out1, times = runner.bench(in_maps1, iters)
        est[r] = _p25(times)
    res["phase1_ns"] = (est[big_rep] - est[1]) / (big_rep - 1) * 1e9
    res["phase1_launch_ns"] = est[1] * 1e9

    pts = [
        np.asarray(o["pt"], np.float16).transpose(1, 0, 2).reshape(AV, B)
        for o in out1
    ]
    in_maps2 = _prep_phase2_inputs(inputs, pts)
    for r in (1, big_rep):
        _, nc2 = _get_programs(mode, r)
        runner = _get_runner(nc2, ("p2", r))
        _, times = runner.bench(in_maps2, iters)
        est[r] = _p25(times)
    res["phase2_ns"] = (est[big_rep] - est[1]) / (big_rep - 1) * 1e9
    res["phase2_launch_ns"] = est[1] * 1e9
    return res


def kernel(**inputs):
    out, _ = _run(inputs)
    return out
